# revision 35
# baseline (speedup 1.0000x reference)
"""Trainium2 Bass kernel for GQA attention block (nn_Attention_20272245637793).

Reference computation (B=2, S=2048, H=2048, 16 q heads / 8 kv heads, D=128):
    q = hs @ Wq.T ; k = hs @ Wk.T ; v = hs @ Wv.T
    rope(q), rope(k); causal softmax(q k^T / sqrt(D)) @ v ; out @ Wo.T

Sharding (8 cores): core i = (b, g) with b = i // 4 (data-parallel over
batch), g = i % 4 (tensor-parallel over kv-head groups; kv heads {2g, 2g+1},
q heads {4g..4g+3}).  Each core computes 1/8 of every GEMM and a partial
o_proj over its 512 head-dims; the host sums the 4 partials per batch.

v2 changes over the first working kernel (trace-driven):
  * 512-wide projection matmuls (BW=512, NB=4) halve per-matmul overhead.
  * Batched 4-chunk DMAs + reordered weight loads cut the ~18us DMA warmup.
  * Scores exp is done per-1024-column PSUM tile (2 banks) instead of
    per-512, cutting ScalarE instruction overhead; phase 2 was exp-bound.
  * Phase 1 tail (q1..q3) and the per-head scores/PV streams are emitted
    interleaved so the in-order PE always has exp-independent work while
    ScalarE drains the softmax (PE stalled behind s_ps backpressure before).
  * attn tile transpose moved from PE (transpose+DVE copy, ~275ns+535ns
    each) to the DMA XBAR (dma transpose, off the critical engines).
  * o_proj PSUM->SBUF copies moved from DVE to ScalarE (idle in phase 3);
    DVE was within 30% of being the bottleneck.

Built on bacc.Bacc: TRN2 instructions can carry at most ONE semaphore wait;
Bacc.compile() legalizes multi-wait instructions.
"""

import sys

sys.path.insert(0, "/opt/trn_rl_repo")

import numpy as np
from contextlib import ExitStack

B = 2
S = 2048
H = 2048
D = 128
NQ = 4          # q heads per core
NKVL = 2        # kv heads per core
HC = H // 128   # 16 h-chunks (contraction)
NB = 4          # hs^T column blocks of 512 for projections
BW = S // NB    # 512
ST = S // 128   # 16 s-tiles / k-chunks / q-tiles
SCALE = 1.0 / np.sqrt(D)

# stripe c of the exp'd transposed scores covers q in [128c, S); offsets of
# the stripes packed into one [128, sum] sbuf tile
STRIPE_LEN = [S - 128 * c for c in range(ST)]
STRIPE_OFF = np.concatenate([[0], np.cumsum(STRIPE_LEN)]).tolist()
PT_TOTAL = STRIPE_OFF[-1]  # 17408

MM_DT = "float16"

_CACHE = {}


def _build_program():
    import concourse.tile as tile
    from concourse import bacc, mybir

    f32 = mybir.dt.float32
    fmm = getattr(mybir.dt, MM_DT)
    nc = bacc.Bacc()

    hsT_d = nc.declare_dram_parameter("hsT", [NB, 128, HC, BW], fmm, isOutput=False)
    wq_d = nc.declare_dram_parameter("wq", [128, HC, 128 * NQ], fmm, isOutput=False)
    wk_d = nc.declare_dram_parameter("wk", [128, HC, 128 * NKVL], fmm, isOutput=False)
    wv_d = nc.declare_dram_parameter("wv", [128, HC, 128 * NKVL], fmm, isOutput=False)
    wo_d = nc.declare_dram_parameter("wo", [128, NQ, H], fmm, isOutput=False)
    cos_d = nc.declare_dram_parameter("cosf", [128, S], fmm, isOutput=False)
    sin_d = nc.declare_dram_parameter("sins", [128, S], fmm, isOutput=False)
    mask_d = nc.declare_dram_parameter("mask", [128, 128], fmm, isOutput=False)
    outT_d = nc.declare_dram_parameter("outT", [H, S], fmm, isOutput=True)

    with tile.TileContext(nc) as tc, ExitStack() as top:
        glob = top.enter_context(tc.tile_pool(name="glob", bufs=1))
        qrot = glob.tile([128, NQ, S], fmm)      # q^T, rope'd, per head
        krot = glob.tile([128, NKVL, S], fmm)    # k^T, rope'd, per kv head
        vaug = glob.tile([128, NKVL, ST, 132], fmm)  # v chunks + ones col @128
        attnT = glob.tile([128, NQ, S], fmm)     # attention out, transposed
        mask_sb = glob.tile([128, 128], fmm)

        nc.vector.memset(vaug[:, :, :, 128:129], 1.0)

        # ---------------- pools (stack allocator: long-lived first) --------
        ph2 = ExitStack()
        ptp = ph2.enter_context(tc.tile_pool(name="p2pt", bufs=2))
        stg = ph2.enter_context(tc.tile_pool(name="p2stg", bufs=12))
        smal = ph2.enter_context(tc.tile_pool(name="p2small", bufs=4))
        ph2s = ExitStack()
        s_ps = ph2s.enter_context(tc.tile_pool(name="p2sps", bufs=3, space="PSUM"))

        ph1 = ExitStack()
        consts = ph1.enter_context(tc.tile_pool(name="p1const", bufs=1))
        hsp = ph1.enter_context(tc.tile_pool(name="p1hs", bufs=2))
        ropep = ph1.enter_context(tc.tile_pool(name="p1rope", bufs=3))
        qk_ps = ph1.enter_context(tc.tile_pool(name="p1qkps", bufs=2, space="PSUM"))

        # ---------------- phase 1 emission helpers ----------------
        def hs_load(nb, split_first=False):
            t = hsp.tile([128, HC, BW], fmm, name=f"hs_{nb}", tag="hs")
            if split_first:
                # land chunk 0 alone first so the very first matmul can start
                nc.sync.dma_start(out=t[:, 0:1, :], in_=hsT_d[nb, :, 0:1, :])
                nc.sync.dma_start(out=t[:, 1:4, :], in_=hsT_d[nb, :, 1:4, :])
            else:
                nc.sync.dma_start(out=t[:, 0:4, :], in_=hsT_d[nb, :, 0:4, :])
            for c4 in range(4, HC, 4):
                nc.sync.dma_start(out=t[:, c4 : c4 + 4, :], in_=hsT_d[nb, :, c4 : c4 + 4, :])
            return t

        # weight / trig loads on the scalar queue, in consumption order
        wq_sb = consts.tile([128, HC, 128 * NQ], fmm)
        wk_sb = consts.tile([128, HC, 128 * NKVL], fmm)
        wv_sb = consts.tile([128, HC, 128 * NKVL], fmm)
        cos_sb = consts.tile([128, S], fmm)
        sin_sb = consts.tile([128, S], fmm)
        hs_tiles = [None] * NB
        hs_tiles[0] = hs_load(0, split_first=True)
        hs_tiles[1] = hs_load(1)
        nc.scalar.dma_start(out=wk_sb[:, 0:1, :], in_=wk_d[:, 0:1, :])
        nc.scalar.dma_start(out=wk_sb[:, 1:4, :], in_=wk_d[:, 1:4, :])
        for c4 in range(4, HC, 4):
            nc.scalar.dma_start(out=wk_sb[:, c4 : c4 + 4, :], in_=wk_d[:, c4 : c4 + 4, :])
        nc.scalar.dma_start(out=cos_sb, in_=cos_d[:, :])
        nc.scalar.dma_start(out=sin_sb, in_=sin_d[:, :])
        nc.scalar.dma_start(out=mask_sb, in_=mask_d[:, :])
        for c4 in range(0, HC, 4):
            nc.scalar.dma_start(out=wv_sb[:, c4 : c4 + 4, :], in_=wv_d[:, c4 : c4 + 4, :])
        for c4 in range(0, HC, 4):
            nc.scalar.dma_start(out=wq_sb[:, c4 : c4 + 4, :], in_=wq_d[:, c4 : c4 + 4, :])

        def qk_tile_gen(nb, mt):
            """mt 0..3 = q heads, 4..5 = k heads. Yields after each matmul."""
            n0 = nb * BW
            hs_t = hs_tiles[nb]
            ps = qk_ps.tile([128, BW], f32, tag="qkps")
            if mt < NQ:
                w_sb, mo = wq_sb, mt
            else:
                w_sb, mo = wk_sb, mt - NQ
            for c in range(HC):
                nc.tensor.matmul(
                    ps,
                    w_sb[:, c, 128 * mo : 128 * mo + 128],
                    hs_t[:, c, :],
                    start=(c == 0),
                    stop=(c == HC - 1),
                )
                yield
            if mt < NQ:
                dest = qrot[:, mt, n0 : n0 + BW]
            else:
                dest = krot[:, mt - NQ, n0 : n0 + BW]
            # rope: dest = ps * cos + swap_halves(ps) * (+/-)sin
            t_t = ropep.tile([128, BW], f32, tag="ropet")
            u_t = ropep.tile([128, BW], f32, tag="ropeu")
            nc.vector.tensor_mul(t_t, ps, cos_sb[:, n0 : n0 + BW])
            nc.vector.tensor_mul(u_t[0:64, :], ps[64:128, :], sin_sb[0:64, n0 : n0 + BW])
            nc.vector.tensor_mul(u_t[64:128, :], ps[0:64, :], sin_sb[64:128, n0 : n0 + BW])
            nc.vector.tensor_add(dest, t_t, u_t)

        def v_tile_gen(nb, st2):
            st = (BW // 128) * nb + st2
            hs_t = hs_tiles[nb]
            psw = qk_ps.tile([128, BW], f32, tag="qkps")
            ps = psw[:, 0 : 128 * NKVL]
            for c in range(HC):
                nc.tensor.matmul(
                    ps,
                    hs_t[:, c, 128 * st2 : 128 * st2 + 128],
                    wv_sb[:, c, :],
                    start=(c == 0),
                    stop=(c == HC - 1),
                )
                yield
            # single strided cast: [128, 2, 128] psum -> vaug[:, :, st, 0:128]
            nc.vector.tensor_copy(
                vaug[:, :, st, 0:128],
                ps.rearrange("p (kv d) -> p kv d", kv=NKVL),
            )

        def run(gen):
            for _ in gen:
                pass

        # ---------------- phase 2 emission helpers ----------------
        def scores_gen(a):
            """scores^T + exp for head a; yields after each sub-matmul."""
            kv = a // 2
            pT = ptp.tile([128, PT_TOTAL], fmm, tag="pT")
            pT_tiles[a] = pT
            for c in range(ST):
                off = STRIPE_OFF[c]
                qlen = STRIPE_LEN[c]
                lhsT = krot[:, kv, 128 * c : 128 * c + 128]
                for blk0 in range(0, qlen, 1024):
                    blkw = min(1024, qlen - blk0)
                    ps = s_ps.tile([128, 1024], f32, tag="sps")
                    for sub0 in range(0, blkw, 512):
                        w = min(512, blkw - sub0)
                        q0 = 128 * c + blk0 + sub0
                        nc.tensor.matmul(
                            ps[:, sub0 : sub0 + w],
                            lhsT,
                            qrot[:, a, q0 : q0 + w],
                            start=True,
                            stop=True,
                        )
                        yield None
                    nc.scalar.activation(
                        pT[:, off + blk0 : off + blk0 + blkw],
                        ps[:, 0:blkw],
                        mybir.ActivationFunctionType.Exp,
                        scale=float(SCALE),
                    )
                # causal mask on the diagonal 128-block of this stripe
                nc.vector.tensor_mul(
                    pT[:, off : off + 128], pT[:, off : off + 128], mask_sb
                )
                yield c
            pT_tiles[a] = pT

        def pv_pair(a, t0, pv_ps):
            """PV + normalize + dma-transpose for tiles t0, t0+1 sharing one
            PSUM bank: chain t0 at cols [0:129], t0+1 at [132:261].  The
            start=True matmul of chain t0 zeroes the whole 2KB zero region,
            so chain t0+1 accumulates with start=False throughout."""
            kv = a // 2
            pT = pT_tiles[a]
            t1 = t0 + 1
            po = pv_ps.tile([128, 261], f32, tag="pvps")
            for c in range(t1 + 1):
                if c <= t0:
                    lhsT = pT[
                        :,
                        STRIPE_OFF[c] + 128 * (t0 - c) : STRIPE_OFF[c] + 128 * (t0 - c) + 128,
                    ]
                    nc.tensor.matmul(
                        po[:, 0:129],
                        lhsT,
                        vaug[:, kv, c, 0:129],
                        start=(c == 0),
                        stop=(c == t0),
                        skip_group_check=True,
                    )
                lhsT = pT[
                    :,
                    STRIPE_OFF[c] + 128 * (t1 - c) : STRIPE_OFF[c] + 128 * (t1 - c) + 128,
                ]
                nc.tensor.matmul(
                    po[:, 132:261],
                    lhsT,
                    vaug[:, kv, c, 0:129],
                    start=False,
                    stop=(c == t1),
                    skip_group_check=True,
                )
            for j, t in ((0, t0), (132, t1)):
                r = smal.tile([128, 1], f32, tag="recip")
                nc.vector.reciprocal(r, po[:, j + 128 : j + 129])
                stage = stg.tile([128, 128], fmm, tag="stage")
                nc.vector.tensor_scalar_mul(stage, po[:, j : j + 128], r)
                nc.sync.dma_start(
                    out=attnT[:, a, 128 * t : 128 * t + 128], in_=stage, transpose=True
                )

        def pv_gen(a, pv_ps):
            for t0 in range(0, ST, 2):
                pv_pair(a, t0, pv_ps)
                yield

        def zip_emit(main_gen, filler_gen, ratio):
            """Drive main_gen; after each yield, pull `ratio` items of filler.
            The debt is primed so filler work is emitted at the window seam,
            where the PE would otherwise wait for the previous head's exp
            to free score-PSUM slots."""
            debt = 8.0 * ratio
            for _ in main_gen:
                debt += ratio
                while debt >= 1.0:
                    if next(filler_gen, _SENT) is _SENT:
                        debt = 0.0
                        break
                    debt -= 1.0
            for _ in filler_gen:
                pass

        _SENT = object()
        pT_tiles = [None] * NQ

        def chain(*gens):
            for g in gens:
                yield from g

        # ---------------- phase 1 blocks 0..2 ----------------
        for nb in range(3):
            if nb >= 1:
                hs_tiles[nb + 1] = hs_load(nb + 1)
            if nb == 0:
                # block 0: k first — its chains consume hs chunks as the
                # slabs land, while V chains need the whole block
                run(qk_tile_gen(nb, 4))
                run(qk_tile_gen(nb, 5))
                for st2 in range(BW // 128):
                    run(v_tile_gen(nb, st2))
            else:
                for st2 in range(BW // 128):
                    run(v_tile_gen(nb, st2))
                run(qk_tile_gen(nb, 4))
                run(qk_tile_gen(nb, 5))
            for mt in range(NQ):
                run(qk_tile_gen(nb, mt))

        # ---------------- block 3: k heads + q0, then interleaved ph2 ------
        run(qk_tile_gen(3, 4))
        run(qk_tile_gen(3, 5))
        run(qk_tile_gen(3, 0))

        # window 0: scores head 0, filled with V(nb3) + q1..q3 projections
        filler0 = chain(
            qk_tile_gen(3, 1),
            qk_tile_gen(3, 2),
            qk_tile_gen(3, 3),
            v_tile_gen(3, 0),
            v_tile_gen(3, 1),
            v_tile_gen(3, 2),
            v_tile_gen(3, 3),
        )
        zip_emit(scores_gen(0), filler0, ratio=2.0)

        # phase-1 pools are drained; free SBUF + PSUM banks
        ph1.close()
        wopstk = ExitStack()
        wop = wopstk.enter_context(tc.tile_pool(name="p3wo", bufs=1))
        wo_sb = wop.tile([128, NQ, H], fmm)
        for a in range(NQ):
            nc.sync.dma_start(out=wo_sb[:, a, :], in_=wo_d[:, a, :])
        pv2stk = ExitStack()
        pv_ps = pv2stk.enter_context(tc.tile_pool(name="p2pvps", bufs=2, space="PSUM"))

        # windows 1,2: scores head a, filled with PV of head a-1
        zip_emit(scores_gen(1), pv_gen(0, pv_ps), ratio=0.15)
        zip_emit(scores_gen(2), pv_gen(1, pv_ps), ratio=0.15)

        # window 3: scores head 3 + PV head 2 filler; PV head-3 tiles are
        # emitted as soon as their pT stripes are exp'd so the XBAR
        # transposes finish before o_proj needs attnT[3]
        pv2g = pv_gen(2, pv_ps)
        pv3_next = 0
        debt = 1.2
        while debt >= 1.0:
            next(pv2g, None)
            debt -= 1.0
        for y in scores_gen(3):
            if isinstance(y, int):
                while pv3_next + 1 <= y and pv3_next < ST:
                    pv_pair(3, pv3_next, pv_ps)
                    pv3_next += 2
            else:
                debt += 0.15
                if debt >= 1.0:
                    next(pv2g, None)
                    debt -= 1.0
        for _ in pv2g:
            pass
        while pv3_next < ST:
            pv_pair(3, pv3_next, pv_ps)
            pv3_next += 2
        pv2stk.close()
        ph2s.close()

        # ---------------- phase 3 o_proj ----------------
        ph3 = ExitStack()
        ostg = ph3.enter_context(tc.tile_pool(name="p3stg", bufs=2))
        o_ps = ph3.enter_context(tc.tile_pool(name="p3ops", bufs=8, space="PSUM"))

        outT_v = outT_d.rearrange("(m p) s -> p m s", p=128)

        def o_block(ns):
            ot = ostg.tile([128, H // 128, 512], fmm, tag="ostg")
            for mt in range(H // 128):
                ps = o_ps.tile([128, 512], f32, tag="ops")
                for a in range(NQ):
                    nc.tensor.matmul(
                        ps,
                        wo_sb[:, a, 128 * mt : 128 * mt + 128],
                        attnT[:, a, 512 * ns : 512 * ns + 512],
                        start=(a == 0),
                        stop=(a == NQ - 1),
                    )
                nc.scalar.copy(ot[:, mt, :], ps)
                if mt % 4 == 3:
                    nc.sync.dma_start(
                        out=outT_v[:, mt - 3 : mt + 1, 512 * ns : 512 * ns + 512],
                        in_=ot[:, mt - 3 : mt + 1, :],
                    )

        for ns in range(S // 512):
            o_block(ns)

        ph3.close()
        wopstk.close()
        ph2.close()

    nc.finalize()
    return nc


def _rope_tables():
    inv_freq = 1.0 / (10000.0 ** (np.arange(0, D, 2, dtype=np.float32) / D))
    t = np.arange(S, dtype=np.float32)[:, None]
    freqs = t * inv_freq[None, :]          # [S, 64]
    cos = np.cos(freqs).astype(np.float32)  # [S, 64]
    sin = np.sin(freqs).astype(np.float32)
    mdt = np.dtype(MM_DT)
    cosf = np.concatenate([cos, cos], axis=1).T.astype(mdt)    # [128, S]
    sins = np.concatenate([-sin, sin], axis=1).T.astype(mdt)   # [128, S]
    return np.ascontiguousarray(cosf), np.ascontiguousarray(sins)


def _prep_in_maps(hidden_states, Wq, Wk, Wv, Wo):
    mdt = np.dtype(MM_DT)
    cosf, sins = _rope_tables()
    mask = np.triu(np.ones((128, 128), dtype=mdt))  # [j, q]: 1 if j <= q

    hsT_blocks = []
    for b in range(B):
        hsT = hidden_states[b].T  # [H, S]
        blk = np.ascontiguousarray(
            hsT.reshape(HC, 128, NB, BW).transpose(2, 1, 0, 3).astype(mdt)
        )  # [NB, 128, HC, BW]
        hsT_blocks.append(blk)

    in_maps = []
    for i in range(8):
        b, g = i // 4, i % 4
        wq = np.ascontiguousarray(
            Wq[512 * g : 512 * (g + 1), :].reshape(512, HC, 128).transpose(2, 1, 0).astype(mdt)
        )
        wk = np.ascontiguousarray(
            Wk[256 * g : 256 * (g + 1), :].reshape(256, HC, 128).transpose(2, 1, 0).astype(mdt)
        )
        wv = np.ascontiguousarray(
            Wv[256 * g : 256 * (g + 1), :].reshape(256, HC, 128).transpose(2, 1, 0).astype(mdt)
        )
        wo = np.ascontiguousarray(
            Wo[:, 512 * g : 512 * (g + 1)].reshape(H, NQ, 128).transpose(2, 1, 0).astype(mdt)
        )
        in_maps.append(
            {
                "hsT": hsT_blocks[b],
                "wq": wq,
                "wk": wk,
                "wv": wv,
                "wo": wo,
                "cosf": cosf,
                "sins": sins,
                "mask": mask,
            }
        )
    return in_maps


def _run(in_maps, **kwargs):
    from concourse.bass_utils import run_bass_kernel_spmd

    if "prog" not in _CACHE:
        _CACHE["prog"] = _build_program()
    nc = _CACHE["prog"]
    return run_bass_kernel_spmd(nc, in_maps, core_ids=list(range(8)), **kwargs)


def _gather(results):
    out = np.empty((B, S, H), dtype=np.float32)
    for b in range(B):
        acc = results[4 * b + 0]["outT"].astype(np.float32)
        for g in range(1, 4):
            acc += results[4 * b + g]["outT"].astype(np.float32)
        out[b] = acc.T
    return out


def kernel(hidden_states, Wq, Wk, Wv, Wo):
    hidden_states = np.asarray(hidden_states, dtype=np.float32)
    Wq = np.asarray(Wq, dtype=np.float32)
    Wk = np.asarray(Wk, dtype=np.float32)
    Wv = np.asarray(Wv, dtype=np.float32)
    Wo = np.asarray(Wo, dtype=np.float32)
    in_maps = _prep_in_maps(hidden_states, Wq, Wk, Wv, Wo)
    res = _run(in_maps)
    return _gather(res.results)


# revision 36
# speedup vs baseline: 1.0164x; 1.0164x over previous
"""Trainium2 Bass kernel for GQA attention block (nn_Attention_20272245637793).

Reference computation (B=2, S=2048, H=2048, 16 q heads / 8 kv heads, D=128):
    q = hs @ Wq.T ; k = hs @ Wk.T ; v = hs @ Wv.T
    rope(q), rope(k); causal softmax(q k^T / sqrt(D)) @ v ; out @ Wo.T

Sharding (8 cores): core i = (b, g) with b = i // 4 (data-parallel over
batch), g = i % 4 (tensor-parallel over kv-head groups; kv heads {2g, 2g+1},
q heads {4g..4g+3}).  Each core computes 1/8 of every GEMM and a partial
o_proj over its 512 head-dims; the host sums the 4 partials per batch.

v2 changes over the first working kernel (trace-driven):
  * 512-wide projection matmuls (BW=512, NB=4) halve per-matmul overhead.
  * Batched 4-chunk DMAs + reordered weight loads cut the ~18us DMA warmup.
  * Scores exp is done per-1024-column PSUM tile (2 banks) instead of
    per-512, cutting ScalarE instruction overhead; phase 2 was exp-bound.
  * Phase 1 tail (q1..q3) and the per-head scores/PV streams are emitted
    interleaved so the in-order PE always has exp-independent work while
    ScalarE drains the softmax (PE stalled behind s_ps backpressure before).
  * attn tile transpose moved from PE (transpose+DVE copy, ~275ns+535ns
    each) to the DMA XBAR (dma transpose, off the critical engines).
  * o_proj PSUM->SBUF copies moved from DVE to ScalarE (idle in phase 3);
    DVE was within 30% of being the bottleneck.

Built on bacc.Bacc: TRN2 instructions can carry at most ONE semaphore wait;
Bacc.compile() legalizes multi-wait instructions.
"""

import sys

sys.path.insert(0, "/opt/trn_rl_repo")

import numpy as np
from contextlib import ExitStack

B = 2
S = 2048
H = 2048
D = 128
NQ = 4          # q heads per core
NKVL = 2        # kv heads per core
HC = H // 128   # 16 h-chunks (contraction)
NB = 4          # hs^T column blocks of 512 for projections
BW = S // NB    # 512
ST = S // 128   # 16 s-tiles / k-chunks / q-tiles
SCALE = 1.0 / np.sqrt(D)

# stripe c of the exp'd transposed scores covers q in [128c, S); offsets of
# the stripes packed into one [128, sum] sbuf tile
STRIPE_LEN = [S - 128 * c for c in range(ST)]
STRIPE_OFF = np.concatenate([[0], np.cumsum(STRIPE_LEN)]).tolist()
PT_TOTAL = STRIPE_OFF[-1]  # 17408

MM_DT = "float16"

_CACHE = {}


def _build_program():
    import concourse.tile as tile
    from concourse import bacc, mybir

    f32 = mybir.dt.float32
    fmm = getattr(mybir.dt, MM_DT)
    nc = bacc.Bacc()

    hsT_d = nc.declare_dram_parameter("hsT", [NB, 128, HC, BW], fmm, isOutput=False)
    wq_d = nc.declare_dram_parameter("wq", [128, HC, 128 * NQ], fmm, isOutput=False)
    wk_d = nc.declare_dram_parameter("wk", [128, HC, 128 * NKVL], fmm, isOutput=False)
    wv_d = nc.declare_dram_parameter("wv", [128, HC, 128 * NKVL], fmm, isOutput=False)
    wo_d = nc.declare_dram_parameter("wo", [128, NQ, H], fmm, isOutput=False)
    cos_d = nc.declare_dram_parameter("cosf", [128, S], fmm, isOutput=False)
    sin_d = nc.declare_dram_parameter("sins", [128, S], fmm, isOutput=False)
    mask_d = nc.declare_dram_parameter("mask", [128, 128], fmm, isOutput=False)
    outT_d = nc.declare_dram_parameter("outT", [H, S], fmm, isOutput=True)

    with tile.TileContext(nc) as tc, ExitStack() as top:
        glob = top.enter_context(tc.tile_pool(name="glob", bufs=1))
        qrot = glob.tile([128, NQ, S], fmm)      # q^T, rope'd, per head
        krot = glob.tile([128, NKVL, S], fmm)    # k^T, rope'd, per kv head
        vaug = glob.tile([128, NKVL, ST, 132], fmm)  # v chunks + ones col @128
        attnT = glob.tile([128, NQ, S], fmm)     # attention out, transposed
        mask_sb = glob.tile([128, 128], fmm)

        nc.vector.memset(vaug[:, :, :, 128:129], 1.0)

        # ---------------- pools (stack allocator: long-lived first) --------
        ph2 = ExitStack()
        ptp = ph2.enter_context(tc.tile_pool(name="p2pt", bufs=2))
        stg = ph2.enter_context(tc.tile_pool(name="p2stg", bufs=12))
        smal = ph2.enter_context(tc.tile_pool(name="p2small", bufs=4))
        ph2s = ExitStack()
        s_ps = ph2s.enter_context(tc.tile_pool(name="p2sps", bufs=3, space="PSUM"))

        ph1 = ExitStack()
        consts = ph1.enter_context(tc.tile_pool(name="p1const", bufs=1))
        hsp = ph1.enter_context(tc.tile_pool(name="p1hs", bufs=2))
        ropep = ph1.enter_context(tc.tile_pool(name="p1rope", bufs=3))
        qk_ps = ph1.enter_context(tc.tile_pool(name="p1qkps", bufs=2, space="PSUM"))

        # ---------------- phase 1 emission helpers ----------------
        def hs_load(nb, split_first=False):
            t = hsp.tile([128, HC, BW], fmm, name=f"hs_{nb}", tag="hs")
            if split_first:
                # land chunk 0 alone first so the very first matmul can start
                nc.sync.dma_start(out=t[:, 0:1, :], in_=hsT_d[nb, :, 0:1, :])
                nc.sync.dma_start(out=t[:, 1:4, :], in_=hsT_d[nb, :, 1:4, :])
            else:
                nc.sync.dma_start(out=t[:, 0:4, :], in_=hsT_d[nb, :, 0:4, :])
            for c4 in range(4, HC, 4):
                nc.sync.dma_start(out=t[:, c4 : c4 + 4, :], in_=hsT_d[nb, :, c4 : c4 + 4, :])
            return t

        # weight / trig loads on the scalar queue, in consumption order
        wq_sb = consts.tile([128, HC, 128 * NQ], fmm)
        wk_sb = consts.tile([128, HC, 128 * NKVL], fmm)
        wv_sb = consts.tile([128, HC, 128 * NKVL], fmm)
        cos_sb = consts.tile([128, S], fmm)
        sin_sb = consts.tile([128, S], fmm)
        hs_tiles = [None] * NB
        hs_tiles[0] = hs_load(0, split_first=True)
        hs_tiles[1] = hs_load(1)
        nc.scalar.dma_start(out=wk_sb[:, 0:1, :], in_=wk_d[:, 0:1, :])
        nc.scalar.dma_start(out=wk_sb[:, 1:4, :], in_=wk_d[:, 1:4, :])
        for c4 in range(4, HC, 4):
            nc.scalar.dma_start(out=wk_sb[:, c4 : c4 + 4, :], in_=wk_d[:, c4 : c4 + 4, :])
        nc.scalar.dma_start(out=cos_sb, in_=cos_d[:, :])
        nc.scalar.dma_start(out=sin_sb, in_=sin_d[:, :])
        nc.scalar.dma_start(out=mask_sb, in_=mask_d[:, :])
        for c4 in range(0, HC, 4):
            nc.scalar.dma_start(out=wv_sb[:, c4 : c4 + 4, :], in_=wv_d[:, c4 : c4 + 4, :])
        for c4 in range(0, HC, 4):
            nc.scalar.dma_start(out=wq_sb[:, c4 : c4 + 4, :], in_=wq_d[:, c4 : c4 + 4, :])

        def qk_tile_gen(nb, mt):
            """mt 0..3 = q heads, 4..5 = k heads. Yields after each matmul."""
            n0 = nb * BW
            hs_t = hs_tiles[nb]
            ps = qk_ps.tile([128, BW], f32, tag="qkps")
            if mt < NQ:
                w_sb, mo = wq_sb, mt
            else:
                w_sb, mo = wk_sb, mt - NQ
            for c in range(HC):
                nc.tensor.matmul(
                    ps,
                    w_sb[:, c, 128 * mo : 128 * mo + 128],
                    hs_t[:, c, :],
                    start=(c == 0),
                    stop=(c == HC - 1),
                )
                yield
            if mt < NQ:
                dest = qrot[:, mt, n0 : n0 + BW]
            else:
                dest = krot[:, mt - NQ, n0 : n0 + BW]
            # rope: dest = ps * cos + swap_halves(ps) * (+/-)sin
            t_t = ropep.tile([128, BW], f32, tag="ropet")
            u_t = ropep.tile([128, BW], f32, tag="ropeu")
            nc.vector.tensor_mul(t_t, ps, cos_sb[:, n0 : n0 + BW])
            nc.vector.tensor_mul(u_t[0:64, :], ps[64:128, :], sin_sb[0:64, n0 : n0 + BW])
            nc.vector.tensor_mul(u_t[64:128, :], ps[0:64, :], sin_sb[64:128, n0 : n0 + BW])
            nc.vector.tensor_add(dest, t_t, u_t)

        def v_tile_gen(nb, st2):
            st = (BW // 128) * nb + st2
            hs_t = hs_tiles[nb]
            psw = s_ps.tile([128, 1024], f32, tag="sps")
            ps = psw[:, 0 : 128 * NKVL]
            for c in range(HC):
                nc.tensor.matmul(
                    ps,
                    hs_t[:, c, 128 * st2 : 128 * st2 + 128],
                    wv_sb[:, c, :],
                    start=(c == 0),
                    stop=(c == HC - 1),
                )
                yield
            # single strided cast: [128, 2, 128] psum -> vaug[:, :, st, 0:128]
            nc.vector.tensor_copy(
                vaug[:, :, st, 0:128],
                ps.rearrange("p (kv d) -> p kv d", kv=NKVL),
            )

        def run(gen):
            for _ in gen:
                pass

        # ---------------- phase 2 emission helpers ----------------
        def scores_gen(a):
            """scores^T + exp for head a; yields after each sub-matmul."""
            kv = a // 2
            pT = ptp.tile([128, PT_TOTAL], fmm, tag="pT")
            pT_tiles[a] = pT
            for c in range(ST):
                off = STRIPE_OFF[c]
                qlen = STRIPE_LEN[c]
                lhsT = krot[:, kv, 128 * c : 128 * c + 128]
                for blk0 in range(0, qlen, 1024):
                    blkw = min(1024, qlen - blk0)
                    ps = s_ps.tile([128, 1024], f32, tag="sps")
                    for sub0 in range(0, blkw, 512):
                        w = min(512, blkw - sub0)
                        q0 = 128 * c + blk0 + sub0
                        nc.tensor.matmul(
                            ps[:, sub0 : sub0 + w],
                            lhsT,
                            qrot[:, a, q0 : q0 + w],
                            start=True,
                            stop=True,
                        )
                        yield None
                    nc.scalar.activation(
                        pT[:, off + blk0 : off + blk0 + blkw],
                        ps[:, 0:blkw],
                        mybir.ActivationFunctionType.Exp,
                        scale=float(SCALE),
                    )
                # causal mask on the diagonal 128-block of this stripe
                nc.vector.tensor_mul(
                    pT[:, off : off + 128], pT[:, off : off + 128], mask_sb
                )
                yield c
            pT_tiles[a] = pT

        def pv_pair(a, t0, pv_ps):
            """PV + normalize + dma-transpose for tiles t0, t0+1 sharing one
            PSUM bank: chain t0 at cols [0:129], t0+1 at [132:261].  The
            start=True matmul of chain t0 zeroes the whole 2KB zero region,
            so chain t0+1 accumulates with start=False throughout."""
            kv = a // 2
            pT = pT_tiles[a]
            t1 = t0 + 1
            po = pv_ps.tile([128, 261], f32, tag="pvps")
            for c in range(t1 + 1):
                if c <= t0:
                    lhsT = pT[
                        :,
                        STRIPE_OFF[c] + 128 * (t0 - c) : STRIPE_OFF[c] + 128 * (t0 - c) + 128,
                    ]
                    nc.tensor.matmul(
                        po[:, 0:129],
                        lhsT,
                        vaug[:, kv, c, 0:129],
                        start=(c == 0),
                        stop=(c == t0),
                        skip_group_check=True,
                    )
                lhsT = pT[
                    :,
                    STRIPE_OFF[c] + 128 * (t1 - c) : STRIPE_OFF[c] + 128 * (t1 - c) + 128,
                ]
                nc.tensor.matmul(
                    po[:, 132:261],
                    lhsT,
                    vaug[:, kv, c, 0:129],
                    start=False,
                    stop=(c == t1),
                    skip_group_check=True,
                )
            for j, t in ((0, t0), (132, t1)):
                r = smal.tile([128, 1], f32, tag="recip")
                nc.vector.reciprocal(r, po[:, j + 128 : j + 129])
                stage = stg.tile([128, 128], fmm, tag="stage")
                nc.vector.tensor_scalar_mul(stage, po[:, j : j + 128], r)
                nc.sync.dma_start(
                    out=attnT[:, a, 128 * t : 128 * t + 128], in_=stage, transpose=True
                )

        def pv_gen(a, pv_ps):
            for t0 in range(0, ST, 2):
                pv_pair(a, t0, pv_ps)
                yield

        def zip_emit(main_gen, filler_gen, ratio):
            """Drive main_gen; after each yield, pull `ratio` items of filler.
            The debt is primed so filler work is emitted at the window seam,
            where the PE would otherwise wait for the previous head's exp
            to free score-PSUM slots."""
            debt = 8.0 * ratio
            for _ in main_gen:
                debt += ratio
                while debt >= 1.0:
                    if next(filler_gen, _SENT) is _SENT:
                        debt = 0.0
                        break
                    debt -= 1.0
            for _ in filler_gen:
                pass

        _SENT = object()
        pT_tiles = [None] * NQ

        def chain(*gens):
            for g in gens:
                yield from g

        # ---------------- phase 1 blocks 0..2 ----------------
        for nb in range(3):
            if nb >= 1:
                hs_tiles[nb + 1] = hs_load(nb + 1)
            run(qk_tile_gen(nb, 4))
            run(qk_tile_gen(nb, 5))
            for st2 in range(BW // 128):
                run(v_tile_gen(nb, st2))
            for mt in range(NQ):
                run(qk_tile_gen(nb, mt))

        # ---------------- block 3: k heads + q0, then interleaved ph2 ------
        run(qk_tile_gen(3, 4))
        run(qk_tile_gen(3, 5))
        run(qk_tile_gen(3, 0))

        # window 0: scores head 0, filled with V(nb3) + q1..q3 projections
        filler0 = chain(
            qk_tile_gen(3, 1),
            qk_tile_gen(3, 2),
            qk_tile_gen(3, 3),
            v_tile_gen(3, 0),
            v_tile_gen(3, 1),
            v_tile_gen(3, 2),
            v_tile_gen(3, 3),
        )
        zip_emit(scores_gen(0), filler0, ratio=2.0)

        # phase-1 pools are drained; free SBUF + PSUM banks
        ph1.close()
        wopstk = ExitStack()
        wop = wopstk.enter_context(tc.tile_pool(name="p3wo", bufs=1))
        wo_sb = wop.tile([128, NQ, H], fmm)
        for a in range(NQ):
            nc.sync.dma_start(out=wo_sb[:, a, :], in_=wo_d[:, a, :])
        pv2stk = ExitStack()
        pv_ps = pv2stk.enter_context(tc.tile_pool(name="p2pvps", bufs=2, space="PSUM"))

        # windows 1,2: scores head a, filled with PV of head a-1
        zip_emit(scores_gen(1), pv_gen(0, pv_ps), ratio=0.15)
        zip_emit(scores_gen(2), pv_gen(1, pv_ps), ratio=0.15)

        # window 3: scores head 3 + PV head 2 filler; PV head-3 tiles are
        # emitted as soon as their pT stripes are exp'd so the XBAR
        # transposes finish before o_proj needs attnT[3]
        pv2g = pv_gen(2, pv_ps)
        pv3_next = 0
        debt = 1.2
        while debt >= 1.0:
            next(pv2g, None)
            debt -= 1.0
        for y in scores_gen(3):
            if isinstance(y, int):
                while pv3_next + 1 <= y and pv3_next < ST:
                    pv_pair(3, pv3_next, pv_ps)
                    pv3_next += 2
            else:
                debt += 0.15
                if debt >= 1.0:
                    next(pv2g, None)
                    debt -= 1.0
        for _ in pv2g:
            pass
        while pv3_next < ST:
            pv_pair(3, pv3_next, pv_ps)
            pv3_next += 2
        pv2stk.close()
        ph2s.close()

        # ---------------- phase 3 o_proj ----------------
        ph3 = ExitStack()
        ostg = ph3.enter_context(tc.tile_pool(name="p3stg", bufs=2))
        o_ps = ph3.enter_context(tc.tile_pool(name="p3ops", bufs=8, space="PSUM"))

        outT_v = outT_d.rearrange("(m p) s -> p m s", p=128)

        def o_block(ns):
            ot = ostg.tile([128, H // 128, 512], fmm, tag="ostg")
            for mt in range(H // 128):
                ps = o_ps.tile([128, 512], f32, tag="ops")
                for a in range(NQ):
                    nc.tensor.matmul(
                        ps,
                        wo_sb[:, a, 128 * mt : 128 * mt + 128],
                        attnT[:, a, 512 * ns : 512 * ns + 512],
                        start=(a == 0),
                        stop=(a == NQ - 1),
                    )
                nc.scalar.copy(ot[:, mt, :], ps)
                if mt % 4 == 3:
                    nc.sync.dma_start(
                        out=outT_v[:, mt - 3 : mt + 1, 512 * ns : 512 * ns + 512],
                        in_=ot[:, mt - 3 : mt + 1, :],
                    )

        for ns in range(S // 512):
            o_block(ns)

        ph3.close()
        wopstk.close()
        ph2.close()

    nc.finalize()
    return nc


def _rope_tables():
    inv_freq = 1.0 / (10000.0 ** (np.arange(0, D, 2, dtype=np.float32) / D))
    t = np.arange(S, dtype=np.float32)[:, None]
    freqs = t * inv_freq[None, :]          # [S, 64]
    cos = np.cos(freqs).astype(np.float32)  # [S, 64]
    sin = np.sin(freqs).astype(np.float32)
    mdt = np.dtype(MM_DT)
    cosf = np.concatenate([cos, cos], axis=1).T.astype(mdt)    # [128, S]
    sins = np.concatenate([-sin, sin], axis=1).T.astype(mdt)   # [128, S]
    return np.ascontiguousarray(cosf), np.ascontiguousarray(sins)


def _prep_in_maps(hidden_states, Wq, Wk, Wv, Wo):
    mdt = np.dtype(MM_DT)
    cosf, sins = _rope_tables()
    mask = np.triu(np.ones((128, 128), dtype=mdt))  # [j, q]: 1 if j <= q

    hsT_blocks = []
    for b in range(B):
        hsT = hidden_states[b].T  # [H, S]
        blk = np.ascontiguousarray(
            hsT.reshape(HC, 128, NB, BW).transpose(2, 1, 0, 3).astype(mdt)
        )  # [NB, 128, HC, BW]
        hsT_blocks.append(blk)

    in_maps = []
    for i in range(8):
        b, g = i // 4, i % 4
        wq = np.ascontiguousarray(
            Wq[512 * g : 512 * (g + 1), :].reshape(512, HC, 128).transpose(2, 1, 0).astype(mdt)
        )
        wk = np.ascontiguousarray(
            Wk[256 * g : 256 * (g + 1), :].reshape(256, HC, 128).transpose(2, 1, 0).astype(mdt)
        )
        wv = np.ascontiguousarray(
            Wv[256 * g : 256 * (g + 1), :].reshape(256, HC, 128).transpose(2, 1, 0).astype(mdt)
        )
        wo = np.ascontiguousarray(
            Wo[:, 512 * g : 512 * (g + 1)].reshape(H, NQ, 128).transpose(2, 1, 0).astype(mdt)
        )
        in_maps.append(
            {
                "hsT": hsT_blocks[b],
                "wq": wq,
                "wk": wk,
                "wv": wv,
                "wo": wo,
                "cosf": cosf,
                "sins": sins,
                "mask": mask,
            }
        )
    return in_maps


def _run(in_maps, **kwargs):
    from concourse.bass_utils import run_bass_kernel_spmd

    if "prog" not in _CACHE:
        _CACHE["prog"] = _build_program()
    nc = _CACHE["prog"]
    return run_bass_kernel_spmd(nc, in_maps, core_ids=list(range(8)), **kwargs)


def _gather(results):
    out = np.empty((B, S, H), dtype=np.float32)
    for b in range(B):
        acc = results[4 * b + 0]["outT"].astype(np.float32)
        for g in range(1, 4):
            acc += results[4 * b + g]["outT"].astype(np.float32)
        out[b] = acc.T
    return out


def kernel(hidden_states, Wq, Wk, Wv, Wo):
    hidden_states = np.asarray(hidden_states, dtype=np.float32)
    Wq = np.asarray(Wq, dtype=np.float32)
    Wk = np.asarray(Wk, dtype=np.float32)
    Wv = np.asarray(Wv, dtype=np.float32)
    Wo = np.asarray(Wo, dtype=np.float32)
    in_maps = _prep_in_maps(hidden_states, Wq, Wk, Wv, Wo)
    res = _run(in_maps)
    return _gather(res.results)


# revision 38
# speedup vs baseline: 1.1687x; 1.1499x over previous
"""Trainium2 Bass kernel for GQA attention block (nn_Attention_20272245637793).

Reference computation (B=2, S=2048, H=2048, 16 q heads / 8 kv heads, D=128):
    q = hs @ Wq.T ; k = hs @ Wk.T ; v = hs @ Wv.T
    rope(q), rope(k); causal softmax(q k^T / sqrt(D)) @ v ; out @ Wo.T

Sharding (8 cores): core i = (b, g) with b = i // 4 (data-parallel over
batch), g = i % 4 (tensor-parallel over kv-head groups; kv heads {2g, 2g+1},
q heads {4g..4g+3}).  Each core computes 1/8 of every GEMM and a partial
o_proj over its 512 head-dims; the host sums the 4 partials per batch.

v2 changes over the first working kernel (trace-driven):
  * 512-wide projection matmuls (BW=512, NB=4) halve per-matmul overhead.
  * Batched 4-chunk DMAs + reordered weight loads cut the ~18us DMA warmup.
  * Scores exp is done per-1024-column PSUM tile (2 banks) instead of
    per-512, cutting ScalarE instruction overhead; phase 2 was exp-bound.
  * Phase 1 tail (q1..q3) and the per-head scores/PV streams are emitted
    interleaved so the in-order PE always has exp-independent work while
    ScalarE drains the softmax (PE stalled behind s_ps backpressure before).
  * attn tile transpose moved from PE (transpose+DVE copy, ~275ns+535ns
    each) to the DMA XBAR (dma transpose, off the critical engines).
  * o_proj PSUM->SBUF copies moved from DVE to ScalarE (idle in phase 3);
    DVE was within 30% of being the bottleneck.

Built on bacc.Bacc: TRN2 instructions can carry at most ONE semaphore wait;
Bacc.compile() legalizes multi-wait instructions.
"""

import sys

sys.path.insert(0, "/opt/trn_rl_repo")

import numpy as np
from contextlib import ExitStack

B = 2
S = 2048
H = 2048
D = 128
NQ = 4          # q heads per core
NKVL = 2        # kv heads per core
HC = H // 128   # 16 h-chunks (contraction)
NB = 4          # hs^T column blocks of 512 for projections
BW = S // NB    # 512
ST = S // 128   # 16 s-tiles / k-chunks / q-tiles
SCALE = 1.0 / np.sqrt(D)

# stripe c of the exp'd transposed scores covers q in [128c, S); offsets of
# the stripes packed into one [128, sum] sbuf tile
STRIPE_LEN = [S - 128 * c for c in range(ST)]
STRIPE_OFF = np.concatenate([[0], np.cumsum(STRIPE_LEN)]).tolist()
PT_TOTAL = STRIPE_OFF[-1]  # 17408

MM_DT = "float16"

_CACHE = {}


def _build_program():
    import concourse.tile as tile
    from concourse import bacc, mybir

    f32 = mybir.dt.float32
    fmm = getattr(mybir.dt, MM_DT)
    nc = bacc.Bacc()

    hsT_d = nc.declare_dram_parameter("hsT", [NB, 128, HC, BW], fmm, isOutput=False)
    wq_d = nc.declare_dram_parameter("wq", [128, HC, 128 * NQ], fmm, isOutput=False)
    wk_d = nc.declare_dram_parameter("wk", [128, HC, 128 * NKVL], fmm, isOutput=False)
    wv_d = nc.declare_dram_parameter("wv", [128, HC, 128 * NKVL], fmm, isOutput=False)
    wo_d = nc.declare_dram_parameter("wo", [128, NQ, H], fmm, isOutput=False)
    cos_d = nc.declare_dram_parameter("cosf", [128, S], fmm, isOutput=False)
    sin_d = nc.declare_dram_parameter("sins", [128, S], fmm, isOutput=False)
    mask_d = nc.declare_dram_parameter("mask", [128, 128], fmm, isOutput=False)
    outT_d = nc.declare_dram_parameter("outT", [H, S], fmm, isOutput=True)

    with tile.TileContext(nc) as tc, ExitStack() as top:
        glob = top.enter_context(tc.tile_pool(name="glob", bufs=1))
        qrot = glob.tile([128, NQ, S], fmm)      # q^T, rope'd, per head
        krot = glob.tile([128, NKVL, S], fmm)    # k^T, rope'd, per kv head
        vaug = glob.tile([128, NKVL, ST, 132], fmm)  # v chunks + ones col @128
        attnT = glob.tile([128, NQ, S], fmm)     # attention out, transposed
        mask_sb = glob.tile([128, 128], fmm)

        nc.vector.memset(vaug[:, :, :, 128:129], 1.0)

        # ---------------- pools (stack allocator: long-lived first) --------
        ph2 = ExitStack()
        ptp = ph2.enter_context(tc.tile_pool(name="p2pt", bufs=2))
        stg = ph2.enter_context(tc.tile_pool(name="p2stg", bufs=12))
        smal = ph2.enter_context(tc.tile_pool(name="p2small", bufs=4))
        ph2s = ExitStack()
        s_ps = ph2s.enter_context(tc.tile_pool(name="p2sps", bufs=3, space="PSUM"))

        ph1 = ExitStack()
        consts = ph1.enter_context(tc.tile_pool(name="p1const", bufs=1))
        hsp = ph1.enter_context(tc.tile_pool(name="p1hs", bufs=2))
        ropep = ph1.enter_context(tc.tile_pool(name="p1rope", bufs=3))
        qk_ps = ph1.enter_context(tc.tile_pool(name="p1qkps", bufs=2, space="PSUM"))

        # ---------------- phase 1 emission helpers ----------------
        def hs_load(nb, split_first=False):
            t = hsp.tile([128, HC, BW], fmm, name=f"hs_{nb}", tag="hs")
            if split_first:
                # land chunk 0 alone first so the very first matmul can start
                nc.sync.dma_start(out=t[:, 0:1, :], in_=hsT_d[nb, :, 0:1, :])
                nc.sync.dma_start(out=t[:, 1:4, :], in_=hsT_d[nb, :, 1:4, :])
            else:
                nc.sync.dma_start(out=t[:, 0:4, :], in_=hsT_d[nb, :, 0:4, :])
            for c4 in range(4, HC, 4):
                nc.sync.dma_start(out=t[:, c4 : c4 + 4, :], in_=hsT_d[nb, :, c4 : c4 + 4, :])
            return t

        # weight / trig loads on the scalar queue, in consumption order
        wq_sb = consts.tile([128, HC, 128 * NQ], fmm)
        wk_sb = consts.tile([128, HC, 128 * NKVL], fmm)
        wv_sb = consts.tile([128, HC, 128 * NKVL], fmm)
        cos_sb = consts.tile([128, S], fmm)
        sin_sb = consts.tile([128, S], fmm)
        hs_tiles = [None] * NB
        hs_tiles[0] = hs_load(0, split_first=True)
        hs_tiles[1] = hs_load(1)
        nc.scalar.dma_start(out=wk_sb[:, 0:1, :], in_=wk_d[:, 0:1, :])
        nc.scalar.dma_start(out=wk_sb[:, 1:4, :], in_=wk_d[:, 1:4, :])
        for c4 in range(4, HC, 4):
            nc.scalar.dma_start(out=wk_sb[:, c4 : c4 + 4, :], in_=wk_d[:, c4 : c4 + 4, :])
        nc.scalar.dma_start(out=cos_sb, in_=cos_d[:, :])
        nc.scalar.dma_start(out=sin_sb, in_=sin_d[:, :])
        nc.scalar.dma_start(out=mask_sb, in_=mask_d[:, :])
        for c4 in range(0, HC, 4):
            nc.scalar.dma_start(out=wv_sb[:, c4 : c4 + 4, :], in_=wv_d[:, c4 : c4 + 4, :])
        for c4 in range(0, HC, 4):
            nc.scalar.dma_start(out=wq_sb[:, c4 : c4 + 4, :], in_=wq_d[:, c4 : c4 + 4, :])

        def qk_tile_gen(nb, mt):
            """mt 0..3 = q heads, 4..5 = k heads. Yields after each matmul."""
            n0 = nb * BW
            hs_t = hs_tiles[nb]
            ps = qk_ps.tile([128, BW], f32, tag="qkps")
            if mt < NQ:
                w_sb, mo = wq_sb, mt
            else:
                w_sb, mo = wk_sb, mt - NQ
            for c in range(HC):
                nc.tensor.matmul(
                    ps,
                    w_sb[:, c, 128 * mo : 128 * mo + 128],
                    hs_t[:, c, :],
                    start=(c == 0),
                    stop=(c == HC - 1),
                )
                yield
            if mt < NQ:
                dest = qrot[:, mt, n0 : n0 + BW]
            else:
                dest = krot[:, mt - NQ, n0 : n0 + BW]
            # rope: dest = ps * cos + swap_halves(ps) * (+/-)sin
            t_t = ropep.tile([128, BW], f32, tag="ropet")
            u_t = ropep.tile([128, BW], f32, tag="ropeu")
            nc.vector.tensor_mul(t_t, ps, cos_sb[:, n0 : n0 + BW])
            nc.vector.tensor_mul(u_t[0:64, :], ps[64:128, :], sin_sb[0:64, n0 : n0 + BW])
            nc.vector.tensor_mul(u_t[64:128, :], ps[0:64, :], sin_sb[64:128, n0 : n0 + BW])
            nc.vector.tensor_add(dest, t_t, u_t)

        def v_tile_gen(nb, st2):
            st = (BW // 128) * nb + st2
            hs_t = hs_tiles[nb]
            psw = s_ps.tile([128, 1024], f32, tag="sps")
            ps = psw[:, 0 : 128 * NKVL]
            for c in range(HC):
                nc.tensor.matmul(
                    ps,
                    hs_t[:, c, 128 * st2 : 128 * st2 + 128],
                    wv_sb[:, c, :],
                    start=(c == 0),
                    stop=(c == HC - 1),
                )
                yield
            # single strided cast: [128, 2, 128] psum -> vaug[:, :, st, 0:128]
            nc.vector.tensor_copy(
                vaug[:, :, st, 0:128],
                ps.rearrange("p (kv d) -> p kv d", kv=NKVL),
            )

        def run(gen):
            for _ in gen:
                pass

        # ---------------- phase 2 emission helpers ----------------
        def scores_gen(a):
            """scores^T + exp for head a; yields after each sub-matmul."""
            kv = a // 2
            pT = ptp.tile([128, PT_TOTAL], fmm, tag="pT")
            pT_tiles[a] = pT
            for c in range(ST):
                off = STRIPE_OFF[c]
                qlen = STRIPE_LEN[c]
                lhsT = krot[:, kv, 128 * c : 128 * c + 128]
                for blk0 in range(0, qlen, 1024):
                    blkw = min(1024, qlen - blk0)
                    ps = s_ps.tile([128, 1024], f32, tag="sps")
                    for sub0 in range(0, blkw, 512):
                        w = min(512, blkw - sub0)
                        q0 = 128 * c + blk0 + sub0
                        nc.tensor.matmul(
                            ps[:, sub0 : sub0 + w],
                            lhsT,
                            qrot[:, a, q0 : q0 + w],
                            start=True,
                            stop=True,
                        )
                        yield None
                    nc.scalar.activation(
                        pT[:, off + blk0 : off + blk0 + blkw],
                        ps[:, 0:blkw],
                        mybir.ActivationFunctionType.Exp,
                        scale=float(SCALE),
                    )
                # causal mask on the diagonal 128-block of this stripe
                nc.vector.tensor_mul(
                    pT[:, off : off + 128], pT[:, off : off + 128], mask_sb
                )
                yield c
            pT_tiles[a] = pT

        def pv_pair(a, t0, pv_ps):
            """PV + normalize + dma-transpose for tiles t0, t0+1 sharing one
            PSUM bank: chain t0 at cols [0:129], t0+1 at [132:261].  The
            start=True matmul of chain t0 zeroes the whole 2KB zero region,
            so chain t0+1 accumulates with start=False throughout."""
            kv = a // 2
            pT = pT_tiles[a]
            t1 = t0 + 1
            po = pv_ps.tile([128, 261], f32, tag="pvps")
            for c in range(t1 + 1):
                if c <= t0:
                    lhsT = pT[
                        :,
                        STRIPE_OFF[c] + 128 * (t0 - c) : STRIPE_OFF[c] + 128 * (t0 - c) + 128,
                    ]
                    nc.tensor.matmul(
                        po[:, 0:129],
                        lhsT,
                        vaug[:, kv, c, 0:129],
                        start=(c == 0),
                        stop=(c == t0),
                        skip_group_check=True,
                    )
                lhsT = pT[
                    :,
                    STRIPE_OFF[c] + 128 * (t1 - c) : STRIPE_OFF[c] + 128 * (t1 - c) + 128,
                ]
                nc.tensor.matmul(
                    po[:, 132:261],
                    lhsT,
                    vaug[:, kv, c, 0:129],
                    start=False,
                    stop=(c == t1),
                    skip_group_check=True,
                )
            for j, t in ((0, t0), (132, t1)):
                r = smal.tile([128, 1], f32, tag="recip")
                nc.vector.reciprocal(r, po[:, j + 128 : j + 129])
                stage = stg.tile([128, 128], fmm, tag="stage")
                nc.vector.tensor_scalar_mul(stage, po[:, j : j + 128], r)
                nc.sync.dma_start(
                    out=attnT[:, a, 128 * t : 128 * t + 128], in_=stage, transpose=True
                )

        def pv_gen(a, pv_ps):
            for t0 in range(0, ST, 2):
                pv_pair(a, t0, pv_ps)
                yield

        def zip_emit(main_gen, filler_gen, ratio):
            """Drive main_gen; after each yield, pull `ratio` items of filler.
            The debt is primed so filler work is emitted at the window seam,
            where the PE would otherwise wait for the previous head's exp
            to free score-PSUM slots."""
            debt = 8.0 * ratio
            for _ in main_gen:
                debt += ratio
                while debt >= 1.0:
                    if next(filler_gen, _SENT) is _SENT:
                        debt = 0.0
                        break
                    debt -= 1.0
            for _ in filler_gen:
                pass

        _SENT = object()
        pT_tiles = [None] * NQ

        def chain(*gens):
            for g in gens:
                yield from g

        # ---------------- phase 1 blocks 0..2 ----------------
        for nb in range(3):
            if nb >= 1:
                hs_tiles[nb + 1] = hs_load(nb + 1)
            run(qk_tile_gen(nb, 4))
            run(qk_tile_gen(nb, 5))
            for st2 in range(BW // 128):
                run(v_tile_gen(nb, st2))
            for mt in range(NQ):
                run(qk_tile_gen(nb, mt))

        # ---------------- block 3: k heads + q0, then interleaved ph2 ------
        run(qk_tile_gen(3, 4))
        run(qk_tile_gen(3, 5))
        run(qk_tile_gen(3, 0))

        # window 0: scores head 0, filled with V(nb3) + q1..q3 projections
        filler0 = chain(
            qk_tile_gen(3, 1),
            qk_tile_gen(3, 2),
            qk_tile_gen(3, 3),
            v_tile_gen(3, 0),
            v_tile_gen(3, 1),
            v_tile_gen(3, 2),
            v_tile_gen(3, 3),
        )
        zip_emit(scores_gen(0), filler0, ratio=2.0)

        # phase-1 pools are drained; free SBUF + PSUM banks
        ph1.close()
        wopstk = ExitStack()
        wop = wopstk.enter_context(tc.tile_pool(name="p3wo", bufs=1))
        wo_sb = wop.tile([128, NQ, H], fmm)
        for a in range(NQ):
            nc.sync.dma_start(out=wo_sb[:, a, :], in_=wo_d[:, a, :])
        pv2stk = ExitStack()
        pv_ps = pv2stk.enter_context(tc.tile_pool(name="p2pvps", bufs=2, space="PSUM"))

        # windows 1,2: scores head a, filled with PV of head a-1
        zip_emit(scores_gen(1), pv_gen(0, pv_ps), ratio=0.15)
        zip_emit(scores_gen(2), pv_gen(1, pv_ps), ratio=0.15)

        # window 3: scores head 3 + PV head 2 filler; PV head-3 tiles are
        # emitted as soon as their pT stripes are exp'd so the XBAR
        # transposes finish before o_proj needs attnT[3]
        pv2g = pv_gen(2, pv_ps)
        pv3_next = 0
        debt = 1.2
        while debt >= 1.0:
            next(pv2g, None)
            debt -= 1.0
        for y in scores_gen(3):
            if isinstance(y, int):
                while pv3_next + 1 <= y and pv3_next < ST:
                    pv_pair(3, pv3_next, pv_ps)
                    pv3_next += 2
            else:
                debt += 0.15
                if debt >= 1.0:
                    next(pv2g, None)
                    debt -= 1.0
        for _ in pv2g:
            pass
        while pv3_next < ST:
            pv_pair(3, pv3_next, pv_ps)
            pv3_next += 2
        pv2stk.close()
        ph2s.close()

        # ---------------- phase 3 o_proj ----------------
        ph3 = ExitStack()
        ostg = ph3.enter_context(tc.tile_pool(name="p3stg", bufs=2))
        o_ps = ph3.enter_context(tc.tile_pool(name="p3ops", bufs=8, space="PSUM"))

        outT_v = outT_d.rearrange("(m p) s -> p m s", p=128)

        def o_block(ns):
            ot = ostg.tile([128, H // 128, 512], fmm, tag="ostg")
            for mt in range(H // 128):
                ps = o_ps.tile([128, 512], f32, tag="ops")
                for a in range(NQ):
                    nc.tensor.matmul(
                        ps,
                        wo_sb[:, a, 128 * mt : 128 * mt + 128],
                        attnT[:, a, 512 * ns : 512 * ns + 512],
                        start=(a == 0),
                        stop=(a == NQ - 1),
                    )
                nc.scalar.copy(ot[:, mt, :], ps)
                if mt % 4 == 3:
                    nc.sync.dma_start(
                        out=outT_v[:, mt - 3 : mt + 1, 512 * ns : 512 * ns + 512],
                        in_=ot[:, mt - 3 : mt + 1, :],
                    )

        for ns in range(S // 512):
            o_block(ns)

        ph3.close()
        wopstk.close()
        ph2.close()

    nc.finalize()
    return nc


def _rope_tables():
    inv_freq = 1.0 / (10000.0 ** (np.arange(0, D, 2, dtype=np.float32) / D))
    t = np.arange(S, dtype=np.float32)[:, None]
    freqs = t * inv_freq[None, :]          # [S, 64]
    cos = np.cos(freqs).astype(np.float32)  # [S, 64]
    sin = np.sin(freqs).astype(np.float32)
    mdt = np.dtype(MM_DT)
    cosf = np.concatenate([cos, cos], axis=1).T.astype(mdt)    # [128, S]
    sins = np.concatenate([-sin, sin], axis=1).T.astype(mdt)   # [128, S]
    return np.ascontiguousarray(cosf), np.ascontiguousarray(sins)


def _prep_in_maps(hidden_states, Wq, Wk, Wv, Wo):
    mdt = np.dtype(MM_DT)
    cosf, sins = _rope_tables()
    mask = np.triu(np.ones((128, 128), dtype=mdt))  # [j, q]: 1 if j <= q

    hsT_blocks = []
    for b in range(B):
        hsT = hidden_states[b].T  # [H, S]
        blk = np.ascontiguousarray(
            hsT.reshape(HC, 128, NB, BW).transpose(2, 1, 0, 3).astype(mdt)
        )  # [NB, 128, HC, BW]
        hsT_blocks.append(blk)

    in_maps = []
    for i in range(8):
        b, g = i // 4, i % 4
        wq = np.ascontiguousarray(
            Wq[512 * g : 512 * (g + 1), :].reshape(512, HC, 128).transpose(2, 1, 0).astype(mdt)
        )
        wk = np.ascontiguousarray(
            Wk[256 * g : 256 * (g + 1), :].reshape(256, HC, 128).transpose(2, 1, 0).astype(mdt)
        )
        wv = np.ascontiguousarray(
            Wv[256 * g : 256 * (g + 1), :].reshape(256, HC, 128).transpose(2, 1, 0).astype(mdt)
        )
        wo = np.ascontiguousarray(
            Wo[:, 512 * g : 512 * (g + 1)].reshape(H, NQ, 128).transpose(2, 1, 0).astype(mdt)
        )
        in_maps.append(
            {
                "hsT": hsT_blocks[b],
                "wq": wq,
                "wk": wk,
                "wv": wv,
                "wo": wo,
                "cosf": cosf,
                "sins": sins,
                "mask": mask,
            }
        )
    return in_maps


def _run(in_maps, **kwargs):
    from concourse.bass_utils import run_bass_kernel_spmd

    if "prog" not in _CACHE:
        _CACHE["prog"] = _build_program()
    nc = _CACHE["prog"]
    return run_bass_kernel_spmd(nc, in_maps, core_ids=list(range(8)), **kwargs)


def _gather(results):
    out = np.empty((B, S, H), dtype=np.float32)
    for b in range(B):
        acc = results[4 * b + 0]["outT"].astype(np.float32)
        for g in range(1, 4):
            acc += results[4 * b + g]["outT"].astype(np.float32)
        out[b] = acc.T
    return out


def kernel(hidden_states, Wq, Wk, Wv, Wo):
    hidden_states = np.asarray(hidden_states, dtype=np.float32)
    Wq = np.asarray(Wq, dtype=np.float32)
    Wk = np.asarray(Wk, dtype=np.float32)
    Wv = np.asarray(Wv, dtype=np.float32)
    Wo = np.asarray(Wo, dtype=np.float32)
    in_maps = _prep_in_maps(hidden_states, Wq, Wk, Wv, Wo)
    res = _run(in_maps)
    return _gather(res.results)


# revision 40
# speedup vs baseline: 1.1744x; 1.0048x over previous
"""Trainium2 Bass kernel for GQA attention block (nn_Attention_20272245637793).

Reference computation (B=2, S=2048, H=2048, 16 q heads / 8 kv heads, D=128):
    q = hs @ Wq.T ; k = hs @ Wk.T ; v = hs @ Wv.T
    rope(q), rope(k); causal softmax(q k^T / sqrt(D)) @ v ; out @ Wo.T

Sharding (8 cores): core i = (b, g) with b = i // 4 (data-parallel over
batch), g = i % 4 (tensor-parallel over kv-head groups; kv heads {2g, 2g+1},
q heads {4g..4g+3}).  Each core computes 1/8 of every GEMM and a partial
o_proj over its 512 head-dims; the host sums the 4 partials per batch.

v2 changes over the first working kernel (trace-driven):
  * 512-wide projection matmuls (BW=512, NB=4) halve per-matmul overhead.
  * Batched 4-chunk DMAs + reordered weight loads cut the ~18us DMA warmup.
  * Scores exp is done per-1024-column PSUM tile (2 banks) instead of
    per-512, cutting ScalarE instruction overhead; phase 2 was exp-bound.
  * Phase 1 tail (q1..q3) and the per-head scores/PV streams are emitted
    interleaved so the in-order PE always has exp-independent work while
    ScalarE drains the softmax (PE stalled behind s_ps backpressure before).
  * attn tile transpose moved from PE (transpose+DVE copy, ~275ns+535ns
    each) to the DMA XBAR (dma transpose, off the critical engines).
  * o_proj PSUM->SBUF copies moved from DVE to ScalarE (idle in phase 3);
    DVE was within 30% of being the bottleneck.

Built on bacc.Bacc: TRN2 instructions can carry at most ONE semaphore wait;
Bacc.compile() legalizes multi-wait instructions.
"""

import sys

sys.path.insert(0, "/opt/trn_rl_repo")

import numpy as np
from contextlib import ExitStack

B = 2
S = 2048
H = 2048
D = 128
NQ = 4          # q heads per core
NKVL = 2        # kv heads per core
HC = H // 128   # 16 h-chunks (contraction)
NB = 4          # hs^T column blocks of 512 for projections
BW = S // NB    # 512
ST = S // 128   # 16 s-tiles / k-chunks / q-tiles
SCALE = 1.0 / np.sqrt(D)

# stripe c of the exp'd transposed scores covers q in [128c, S); offsets of
# the stripes packed into one [128, sum] sbuf tile
STRIPE_LEN = [S - 128 * c for c in range(ST)]
STRIPE_OFF = np.concatenate([[0], np.cumsum(STRIPE_LEN)]).tolist()
PT_TOTAL = STRIPE_OFF[-1]  # 17408

MM_DT = "float16"

_CACHE = {}


def _build_program():
    import concourse.tile as tile
    from concourse import bacc, mybir

    f32 = mybir.dt.float32
    fmm = getattr(mybir.dt, MM_DT)
    nc = bacc.Bacc()

    hsT_d = nc.declare_dram_parameter("hsT", [NB, 128, HC, BW], fmm, isOutput=False)
    wq_d = nc.declare_dram_parameter("wq", [128, HC, 128 * NQ], fmm, isOutput=False)
    wk_d = nc.declare_dram_parameter("wk", [128, HC, 128 * NKVL], fmm, isOutput=False)
    wv_d = nc.declare_dram_parameter("wv", [128, HC, 128 * NKVL], fmm, isOutput=False)
    wo_d = nc.declare_dram_parameter("wo", [128, NQ, H], fmm, isOutput=False)
    cos_d = nc.declare_dram_parameter("cosf", [128, S], fmm, isOutput=False)
    sin_d = nc.declare_dram_parameter("sins", [128, S], fmm, isOutput=False)
    mask_d = nc.declare_dram_parameter("mask", [128, 128], fmm, isOutput=False)
    outT_d = nc.declare_dram_parameter("outT", [H, S], fmm, isOutput=True)

    with tile.TileContext(nc) as tc, ExitStack() as top:
        glob = top.enter_context(tc.tile_pool(name="glob", bufs=1))
        qrot = glob.tile([128, NQ, S], fmm)      # q^T, rope'd, per head
        krot = glob.tile([128, NKVL, S], fmm)    # k^T, rope'd, per kv head
        vaug = glob.tile([128, NKVL, ST, 132], fmm)  # v chunks + ones col @128
        attnT = glob.tile([128, NQ, S], fmm)     # attention out, transposed
        mask_sb = glob.tile([128, 128], fmm)

        nc.vector.memset(vaug[:, :, :, 128:129], 1.0)

        # ---------------- pools (stack allocator: long-lived first) --------
        ph2 = ExitStack()
        ptp = ph2.enter_context(tc.tile_pool(name="p2pt", bufs=2))
        stg = ph2.enter_context(tc.tile_pool(name="p2stg", bufs=12))
        smal = ph2.enter_context(tc.tile_pool(name="p2small", bufs=4))
        ph2s = ExitStack()
        s_ps = ph2s.enter_context(tc.tile_pool(name="p2sps", bufs=3, space="PSUM"))

        ph1 = ExitStack()
        consts = ph1.enter_context(tc.tile_pool(name="p1const", bufs=1))
        hsp = ph1.enter_context(tc.tile_pool(name="p1hs", bufs=2))
        ropep = ph1.enter_context(tc.tile_pool(name="p1rope", bufs=3))
        qk_ps = ph1.enter_context(tc.tile_pool(name="p1qkps", bufs=2, space="PSUM"))

        # ---------------- phase 1 emission helpers ----------------
        def hs_load(nb, split_first=False):
            t = hsp.tile([128, HC, BW], fmm, name=f"hs_{nb}", tag="hs")
            if split_first:
                # land chunk 0 alone first so the very first matmul can start
                nc.sync.dma_start(out=t[:, 0:1, :], in_=hsT_d[nb, :, 0:1, :])
                nc.sync.dma_start(out=t[:, 1:4, :], in_=hsT_d[nb, :, 1:4, :])
            else:
                nc.sync.dma_start(out=t[:, 0:4, :], in_=hsT_d[nb, :, 0:4, :])
            for c4 in range(4, HC, 4):
                nc.sync.dma_start(out=t[:, c4 : c4 + 4, :], in_=hsT_d[nb, :, c4 : c4 + 4, :])
            return t

        # weight / trig loads on the scalar queue, in consumption order
        wq_sb = consts.tile([128, HC, 128 * NQ], fmm)
        wk_sb = consts.tile([128, HC, 128 * NKVL], fmm)
        wv_sb = consts.tile([128, HC, 128 * NKVL], fmm)
        cos_sb = consts.tile([128, S], fmm)
        sin_sb = consts.tile([128, S], fmm)
        hs_tiles = [None] * NB
        hs_tiles[0] = hs_load(0, split_first=True)
        hs_tiles[1] = hs_load(1)
        nc.scalar.dma_start(out=wk_sb[:, 0:1, :], in_=wk_d[:, 0:1, :])
        nc.scalar.dma_start(out=wk_sb[:, 1:4, :], in_=wk_d[:, 1:4, :])
        for c4 in range(4, HC, 4):
            nc.scalar.dma_start(out=wk_sb[:, c4 : c4 + 4, :], in_=wk_d[:, c4 : c4 + 4, :])
        nc.scalar.dma_start(out=cos_sb, in_=cos_d[:, :])
        nc.scalar.dma_start(out=sin_sb, in_=sin_d[:, :])
        nc.scalar.dma_start(out=mask_sb, in_=mask_d[:, :])
        for c4 in range(0, HC, 4):
            nc.scalar.dma_start(out=wv_sb[:, c4 : c4 + 4, :], in_=wv_d[:, c4 : c4 + 4, :])
        for c4 in range(0, HC, 4):
            nc.scalar.dma_start(out=wq_sb[:, c4 : c4 + 4, :], in_=wq_d[:, c4 : c4 + 4, :])

        def qk_tile_gen(nb, mt):
            """mt 0..3 = q heads, 4..5 = k heads. Yields after each matmul."""
            n0 = nb * BW
            hs_t = hs_tiles[nb]
            ps = qk_ps.tile([128, BW], f32, tag="qkps")
            if mt < NQ:
                w_sb, mo = wq_sb, mt
            else:
                w_sb, mo = wk_sb, mt - NQ
            for c in range(HC):
                nc.tensor.matmul(
                    ps,
                    w_sb[:, c, 128 * mo : 128 * mo + 128],
                    hs_t[:, c, :],
                    start=(c == 0),
                    stop=(c == HC - 1),
                )
                yield
            if mt < NQ:
                dest = qrot[:, mt, n0 : n0 + BW]
            else:
                dest = krot[:, mt - NQ, n0 : n0 + BW]
            # rope: dest = ps * cos + swap_halves(ps) * (+/-)sin
            t_t = ropep.tile([128, BW], f32, tag="ropet")
            u_t = ropep.tile([128, BW], f32, tag="ropeu")
            nc.vector.tensor_mul(t_t, ps, cos_sb[:, n0 : n0 + BW])
            nc.vector.tensor_mul(u_t[0:64, :], ps[64:128, :], sin_sb[0:64, n0 : n0 + BW])
            nc.vector.tensor_mul(u_t[64:128, :], ps[0:64, :], sin_sb[64:128, n0 : n0 + BW])
            nc.vector.tensor_add(dest, t_t, u_t)

        def v_tile_gen(nb, st2):
            st = (BW // 128) * nb + st2
            hs_t = hs_tiles[nb]
            psw = s_ps.tile([128, 1024], f32, tag="sps")
            ps = psw[:, 0 : 128 * NKVL]
            for c in range(HC):
                nc.tensor.matmul(
                    ps,
                    hs_t[:, c, 128 * st2 : 128 * st2 + 128],
                    wv_sb[:, c, :],
                    start=(c == 0),
                    stop=(c == HC - 1),
                )
                yield
            # single strided cast: [128, 2, 128] psum -> vaug[:, :, st, 0:128]
            nc.vector.tensor_copy(
                vaug[:, :, st, 0:128],
                ps.rearrange("p (kv d) -> p kv d", kv=NKVL),
            )

        def run(gen):
            for _ in gen:
                pass

        # ---------------- phase 2 emission helpers ----------------
        def scores_gen(a):
            """scores^T + exp for head a; yields after each sub-matmul."""
            kv = a // 2
            pT = ptp.tile([128, PT_TOTAL], fmm, tag="pT")
            pT_tiles[a] = pT
            for c in range(ST):
                off = STRIPE_OFF[c]
                qlen = STRIPE_LEN[c]
                lhsT = krot[:, kv, 128 * c : 128 * c + 128]
                for blk0 in range(0, qlen, 1024):
                    blkw = min(1024, qlen - blk0)
                    ps = s_ps.tile([128, 1024], f32, tag="sps")
                    for sub0 in range(0, blkw, 512):
                        w = min(512, blkw - sub0)
                        q0 = 128 * c + blk0 + sub0
                        nc.tensor.matmul(
                            ps[:, sub0 : sub0 + w],
                            lhsT,
                            qrot[:, a, q0 : q0 + w],
                            start=True,
                            stop=True,
                        )
                        yield None
                    nc.scalar.activation(
                        pT[:, off + blk0 : off + blk0 + blkw],
                        ps[:, 0:blkw],
                        mybir.ActivationFunctionType.Exp,
                        scale=float(SCALE),
                    )
                # causal mask on the diagonal 128-block of this stripe
                nc.vector.tensor_mul(
                    pT[:, off : off + 128], pT[:, off : off + 128], mask_sb
                )
                yield c
            pT_tiles[a] = pT

        def pv_pair(a, t0, pv_ps):
            """PV + normalize + dma-transpose for tiles t0, t0+1 sharing one
            PSUM bank: chain t0 at cols [0:129], t0+1 at [132:261].  The
            start=True matmul of chain t0 zeroes the whole 2KB zero region,
            so chain t0+1 accumulates with start=False throughout."""
            kv = a // 2
            pT = pT_tiles[a]
            t1 = t0 + 1
            po = pv_ps.tile([128, 261], f32, tag="pvps")
            for c in range(t1 + 1):
                if c <= t0:
                    lhsT = pT[
                        :,
                        STRIPE_OFF[c] + 128 * (t0 - c) : STRIPE_OFF[c] + 128 * (t0 - c) + 128,
                    ]
                    nc.tensor.matmul(
                        po[:, 0:129],
                        lhsT,
                        vaug[:, kv, c, 0:129],
                        start=(c == 0),
                        stop=(c == t0),
                        skip_group_check=True,
                    )
                lhsT = pT[
                    :,
                    STRIPE_OFF[c] + 128 * (t1 - c) : STRIPE_OFF[c] + 128 * (t1 - c) + 128,
                ]
                nc.tensor.matmul(
                    po[:, 132:261],
                    lhsT,
                    vaug[:, kv, c, 0:129],
                    start=False,
                    stop=(c == t1),
                    skip_group_check=True,
                )
            for j, t in ((0, t0), (132, t1)):
                r = smal.tile([128, 1], f32, tag="recip")
                nc.vector.reciprocal(r, po[:, j + 128 : j + 129])
                stage = stg.tile([128, 128], fmm, tag="stage")
                nc.vector.tensor_scalar_mul(stage, po[:, j : j + 128], r)
                nc.sync.dma_start(
                    out=attnT[:, a, 128 * t : 128 * t + 128], in_=stage, transpose=True
                )

        def pv_gen(a, pv_ps):
            for t0 in range(0, ST, 2):
                pv_pair(a, t0, pv_ps)
                yield

        def zip_emit(main_gen, filler_gen, ratio):
            """Drive main_gen; after each yield, pull `ratio` items of filler.
            The debt is primed so filler work is emitted at the window seam,
            where the PE would otherwise wait for the previous head's exp
            to free score-PSUM slots."""
            debt = 8.0 * ratio
            for _ in main_gen:
                debt += ratio
                while debt >= 1.0:
                    if next(filler_gen, _SENT) is _SENT:
                        debt = 0.0
                        break
                    debt -= 1.0
            for _ in filler_gen:
                pass

        _SENT = object()
        pT_tiles = [None] * NQ

        def chain(*gens):
            for g in gens:
                yield from g

        # ---------------- phase 1 blocks 0..2 ----------------
        for nb in range(3):
            if nb >= 1:
                hs_tiles[nb + 1] = hs_load(nb + 1)
            run(qk_tile_gen(nb, 4))
            run(qk_tile_gen(nb, 5))
            for st2 in range(BW // 128):
                run(v_tile_gen(nb, st2))
            for mt in range(NQ):
                run(qk_tile_gen(nb, mt))

        # ---------------- block 3: k heads + q0, then interleaved ph2 ------
        run(qk_tile_gen(3, 4))
        run(qk_tile_gen(3, 5))
        run(qk_tile_gen(3, 0))

        # window 0: scores head 0, filled with V(nb3) + q1..q3 projections
        filler0 = chain(
            qk_tile_gen(3, 1),
            qk_tile_gen(3, 2),
            qk_tile_gen(3, 3),
            v_tile_gen(3, 0),
            v_tile_gen(3, 1),
            v_tile_gen(3, 2),
            v_tile_gen(3, 3),
        )
        zip_emit(scores_gen(0), filler0, ratio=2.0)

        # phase-1 pools are drained; free SBUF + PSUM banks
        ph1.close()
        wopstk = ExitStack()
        wop = wopstk.enter_context(tc.tile_pool(name="p3wo", bufs=1))
        wo_sb = wop.tile([128, NQ, H], fmm)
        for a in range(NQ):
            nc.sync.dma_start(out=wo_sb[:, a, :], in_=wo_d[:, a, :])
        pv2stk = ExitStack()
        pv_ps = pv2stk.enter_context(tc.tile_pool(name="p2pvps", bufs=2, space="PSUM"))

        # windows 1,2: scores head a, filled with PV of head a-1
        zip_emit(scores_gen(1), pv_gen(0, pv_ps), ratio=0.15)
        zip_emit(scores_gen(2), pv_gen(1, pv_ps), ratio=0.15)

        # window 3: scores head 3 + PV head 2 filler; PV head-3 tiles are
        # emitted as soon as their pT stripes are exp'd so the XBAR
        # transposes finish before o_proj needs attnT[3]
        pv2g = pv_gen(2, pv_ps)
        pv3_next = 0
        debt = 1.2
        while debt >= 1.0:
            next(pv2g, None)
            debt -= 1.0
        for y in scores_gen(3):
            if isinstance(y, int):
                while pv3_next + 1 <= y and pv3_next < ST:
                    pv_pair(3, pv3_next, pv_ps)
                    pv3_next += 2
            else:
                debt += 0.15
                if debt >= 1.0:
                    next(pv2g, None)
                    debt -= 1.0
        for _ in pv2g:
            pass
        while pv3_next < ST:
            pv_pair(3, pv3_next, pv_ps)
            pv3_next += 2
        pv2stk.close()
        ph2s.close()

        # ---------------- phase 3 o_proj ----------------
        ph3 = ExitStack()
        ostg = ph3.enter_context(tc.tile_pool(name="p3stg", bufs=2))
        o_ps = ph3.enter_context(tc.tile_pool(name="p3ops", bufs=8, space="PSUM"))

        outT_v = outT_d.rearrange("(m p) s -> p m s", p=128)

        def o_block(ns):
            ot = ostg.tile([128, H // 128, 512], fmm, tag="ostg")
            for mt in range(H // 128):
                ps = o_ps.tile([128, 512], f32, tag="ops")
                for a in range(NQ):
                    nc.tensor.matmul(
                        ps,
                        wo_sb[:, a, 128 * mt : 128 * mt + 128],
                        attnT[:, a, 512 * ns : 512 * ns + 512],
                        start=(a == 0),
                        stop=(a == NQ - 1),
                    )
                nc.scalar.copy(ot[:, mt, :], ps)
                if mt % 4 == 3:
                    nc.sync.dma_start(
                        out=outT_v[:, mt - 3 : mt + 1, 512 * ns : 512 * ns + 512],
                        in_=ot[:, mt - 3 : mt + 1, :],
                    )

        for ns in range(S // 512):
            o_block(ns)

        ph3.close()
        wopstk.close()
        ph2.close()

    nc.finalize()
    return nc


def _rope_tables():
    inv_freq = 1.0 / (10000.0 ** (np.arange(0, D, 2, dtype=np.float32) / D))
    t = np.arange(S, dtype=np.float32)[:, None]
    freqs = t * inv_freq[None, :]          # [S, 64]
    cos = np.cos(freqs).astype(np.float32)  # [S, 64]
    sin = np.sin(freqs).astype(np.float32)
    mdt = np.dtype(MM_DT)
    cosf = np.concatenate([cos, cos], axis=1).T.astype(mdt)    # [128, S]
    sins = np.concatenate([-sin, sin], axis=1).T.astype(mdt)   # [128, S]
    return np.ascontiguousarray(cosf), np.ascontiguousarray(sins)


def _prep_in_maps(hidden_states, Wq, Wk, Wv, Wo):
    mdt = np.dtype(MM_DT)
    cosf, sins = _rope_tables()
    mask = np.triu(np.ones((128, 128), dtype=mdt))  # [j, q]: 1 if j <= q

    hsT_blocks = []
    for b in range(B):
        hsT = hidden_states[b].T  # [H, S]
        blk = np.ascontiguousarray(
            hsT.reshape(HC, 128, NB, BW).transpose(2, 1, 0, 3).astype(mdt)
        )  # [NB, 128, HC, BW]
        hsT_blocks.append(blk)

    in_maps = []
    for i in range(8):
        b, g = i // 4, i % 4
        wq = np.ascontiguousarray(
            Wq[512 * g : 512 * (g + 1), :].reshape(512, HC, 128).transpose(2, 1, 0).astype(mdt)
        )
        wk = np.ascontiguousarray(
            Wk[256 * g : 256 * (g + 1), :].reshape(256, HC, 128).transpose(2, 1, 0).astype(mdt)
        )
        wv = np.ascontiguousarray(
            Wv[256 * g : 256 * (g + 1), :].reshape(256, HC, 128).transpose(2, 1, 0).astype(mdt)
        )
        wo = np.ascontiguousarray(
            Wo[:, 512 * g : 512 * (g + 1)].reshape(H, NQ, 128).transpose(2, 1, 0).astype(mdt)
        )
        in_maps.append(
            {
                "hsT": hsT_blocks[b],
                "wq": wq,
                "wk": wk,
                "wv": wv,
                "wo": wo,
                "cosf": cosf,
                "sins": sins,
                "mask": mask,
            }
        )
    return in_maps


def _run(in_maps, **kwargs):
    from concourse.bass_utils import run_bass_kernel_spmd

    if "prog" not in _CACHE:
        _CACHE["prog"] = _build_program()
    nc = _CACHE["prog"]
    return run_bass_kernel_spmd(nc, in_maps, core_ids=list(range(8)), **kwargs)


def _gather(results):
    out = np.empty((B, S, H), dtype=np.float32)
    for b in range(B):
        acc = results[4 * b + 0]["outT"].astype(np.float32)
        for g in range(1, 4):
            acc += results[4 * b + g]["outT"].astype(np.float32)
        out[b] = acc.T
    return out


def kernel(hidden_states, Wq, Wk, Wv, Wo):
    hidden_states = np.asarray(hidden_states, dtype=np.float32)
    Wq = np.asarray(Wq, dtype=np.float32)
    Wk = np.asarray(Wk, dtype=np.float32)
    Wv = np.asarray(Wv, dtype=np.float32)
    Wo = np.asarray(Wo, dtype=np.float32)
    in_maps = _prep_in_maps(hidden_states, Wq, Wk, Wv, Wo)
    res = _run(in_maps)
    return _gather(res.results)
